# revision 19
# baseline (speedup 1.0000x reference)
"""MultiHead Differential Attention on 8 Trainium2 NeuronCores.

Sharding: data-parallel over batch (B=2), tensor-parallel over heads
(16 heads -> 4 per core).  Core c handles batch c//4, heads (c%4)*4..+4.

Device layout is fully "transposed" (S^T = [k, q] orientation) so that no
on-device transposes are ever needed:
  - projections compute Q^T, K^T directly ([2*Dh, seq]); V naturally [seq, dv]
  - S^T[k,q] = K^T.T @ Q^T  (contraction over d on partitions)
  - softmax row-sums come from an all-ones stationary matmul (M=128 -> the
    sums arrive pre-broadcast across partitions)
  - PV keeps V stationary: O^T[dv,q] accumulates over k-chunks
  - combine/RMS happen on broadcast tiles; out-proj streams O^T with Wo
    stationary, producing OUT^T which the host transposes and sum-reduces.
"""
import numpy as np
import ml_dtypes
from contextlib import ExitStack

import concourse.bass as bass
import concourse.mybir as mybir
import concourse.tile as tile
from concourse import bacc
from concourse.bass_utils import run_bass_kernel_spmd

BF16 = mybir.dt.bfloat16
F32 = mybir.dt.float32
AF = mybir.ActivationFunctionType
ALU = mybir.AluOpType

D_MODEL = 1024
H = 16
DH = 64          # head dim per component
HD = 2 * DH      # 128, per-head width of Q/K/V
N = 2048         # sequence length
B = 2
HPC = 4          # heads per core
LAMBDA_INIT = 0.8
EPS = 1e-5
SCALING = 1.0 / np.sqrt(DH)

MC = D_MODEL // 128   # 8 contraction chunks for projections
QC = 4                # q chunks of 512
KCQ = 4               # k-chunks (128) per q chunk
NKC = 16              # total k chunks

_cache = {}


def _patch_act_tables():
    """Force Exp and Ln to resolve to the single set that contains both,
    so alternating Exp/Ln never reloads activation tables."""
    import concourse.bacc as bacc_mod
    import concourse.hw_specs as hw_specs_mod
    if getattr(bacc_mod, "_act_tables_patched", False):
        return
    orig = hw_specs_mod.get_activation_tables

    def patched(arch):
        t = orig(arch)
        for name, fns in t.items():
            if name != "natural_log_exp_and_others":
                fns.discard(AF.Exp)
                fns.discard(AF.Ln)
        return t

    bacc_mod.get_activation_tables = patched
    bacc_mod._act_tables_patched = True


def _build():
    _patch_act_tables()
    nc = bacc.Bacc("TRN2", target_bir_lowering=False, debug=False)

    xt_d = nc.dram_tensor("xt", [128, MC, N], BF16, kind="ExternalInput").ap()
    wq_d = nc.dram_tensor("wq", [128, MC, HPC * HD], BF16, kind="ExternalInput").ap()
    wk_d = nc.dram_tensor("wk", [128, MC, HPC * HD], BF16, kind="ExternalInput").ap()
    wv_d = nc.dram_tensor("wv", [128, MC, HPC * HD], BF16, kind="ExternalInput").ap()
    wo_d = nc.dram_tensor("wo", [128, HPC, 8, 128], BF16, kind="ExternalInput").ap()
    lam_d = nc.dram_tensor("lam", [128, 4 * HPC], F32, kind="ExternalInput").ap()
    msk_d = nc.dram_tensor("msk", [128, KCQ, 512], BF16, kind="ExternalInput").ap()
    out_d = nc.dram_tensor("outT", [D_MODEL, N], F32, kind="ExternalOutput").ap()

    with tile.TileContext(nc) as tc, ExitStack() as ctx:
        # ---- long-lived tiles
        keep = ctx.enter_context(tc.tile_pool(name="keep", bufs=1))
        qt = [keep.tile([128, N], BF16, tag=f"qt{h}", name=f"qt{h}") for h in range(HPC)]
        kt = [keep.tile([128, N], BF16, tag=f"kt{h}", name=f"kt{h}") for h in range(HPC)]
        vb = keep.tile([128, NKC, 512], BF16, tag="vb")
        otf = [keep.tile([128, N], BF16, tag=f"otf{h}", name=f"otf{h}") for h in range(HPC)]
        lam_t = keep.tile([128, 4 * HPC], F32, tag="lam")
        msk_t = keep.tile([128, KCQ, 512], BF16, tag="msk")
        ones_t = keep.tile([128, 128], BF16, tag="ones")
        eps_t = keep.tile([128, 1], F32, tag="eps")
        wo_t = keep.tile([128, HPC, 8, 128], BF16, tag="wo")

        nc.sync.dma_start(lam_t[:], lam_d[:])
        nc.sync.dma_start(msk_t[:], msk_d[:])
        nc.sync.dma_start(wo_t[:], wo_d[:])
        nc.gpsimd.memset(ones_t[:], 1.0)
        nc.gpsimd.memset(eps_t[:], float(EPS))

        # ================= phase 1: projections =================
        pj = ctx.enter_context(tc.tile_pool(name="proj", bufs=1))
        psum = ctx.enter_context(tc.tile_pool(name="psum", bufs=1, space="PSUM"))
        at = ctx.enter_context(tc.tile_pool(name="att", bufs=2))
        ep = ctx.enter_context(tc.tile_pool(name="esb", bufs=2))
        osb = ctx.enter_context(tc.tile_pool(name="osb", bufs=2))
        if True:
            xtb = pj.tile([128, MC, N], BF16, tag="xtb")
            wqb = pj.tile([128, MC, HPC * HD], BF16, tag="wqb")
            wkb = pj.tile([128, MC, HPC * HD], BF16, tag="wkb")
            wvb = pj.tile([128, MC, HPC * HD], BF16, tag="wvb")
            for mc in range(MC):
                nc.sync.dma_start(xtb[:, mc, :], xt_d[:, mc, :])
                nc.sync.dma_start(wvb[:, mc, :], wv_d[:, mc, :])
                nc.sync.dma_start(wqb[:, mc, :], wq_d[:, mc, :])
                nc.sync.dma_start(wkb[:, mc, :], wk_d[:, mc, :])

            # interleaved: V seq-chunks for qc, then all heads' QT/KT qc-chunk,
            # so attention(qc=0) unblocks as early as possible
            for qc in range(QC):
                for sc in range(4 * qc, 4 * qc + 4):
                    ps = psum.tile([128, 2, 512], F32, tag="sg", name="vps", bufs=2)
                    for mc in range(MC):
                        nc.tensor.matmul(
                            ps[:, 0, :],
                            xtb[:, mc, sc * 128:(sc + 1) * 128],
                            wvb[:, mc, :],
                            start=(mc == 0), stop=(mc == MC - 1))
                    nc.scalar.copy(vb[:, sc, :], ps[:, 0, :])
                for h in range(HPC):
                    for (wsrc, dst) in ((wqb, qt[h]), (wkb, kt[h])):
                        ps = psum.tile([128, 2, 512], F32, tag="sg", name="qkps", bufs=2)
                        for mc in range(MC):
                            nc.tensor.matmul(
                                ps[:, 0, :],
                                wsrc[:, mc, h * HD:(h + 1) * HD],
                                xtb[:, mc, qc * 512:(qc + 1) * 512],
                                start=(mc == 0), stop=(mc == MC - 1))
                        nc.scalar.copy(dst[:, qc * 512:(qc + 1) * 512], ps[:, 0, :])

        # ================= phase 2: attention =================
        if True:
            for qc in range(QC):
                for h in range(HPC):
                    nkc = KCQ * qc + KCQ  # k chunks in play
                    q0 = qc * 512
                    s1bc = psum.tile([128, 512], F32, tag="s1bc")
                    s2bc = psum.tile([128, 512], F32, tag="s2bc")
                    o1 = psum.tile([128, 512], F32, tag="o1")
                    o2 = psum.tile([128, 512], F32, tag="o2")
                    ngrp = (nkc + 1) // 2
                    hold = [None]
                    pending = []

                    def emit_b(item):
                        e1, e2, kcs_b, g = item
                        full_pair = (2 * g + 1 < KCQ * qc)
                        if full_pair:
                            ep1 = ep.tile([128, 512], BF16, tag="ep1", name="ep1", bufs=3)
                            nc.gpsimd.tensor_add(ep1[:], e1[:, 0, :], e1[:, 1, :])
                            ep2 = ep.tile([128, 512], BF16, tag="ep2", name="ep2", bufs=3)
                            nc.gpsimd.tensor_add(ep2[:], e2[:, 0, :], e2[:, 1, :])
                            if g % 2 == 0 and 2 * (g + 1) + 1 < KCQ * qc:
                                hold[0] = (ep1, ep2)  # fold into partner pair
                            elif g % 2 == 1 and hold[0] is not None:
                                q1 = ep.tile([128, 512], BF16, tag="q1", name="q1")
                                nc.gpsimd.tensor_add(q1[:], hold[0][0][:], ep1[:])
                                q2 = ep.tile([128, 512], BF16, tag="q2", name="q2")
                                nc.gpsimd.tensor_add(q2[:], hold[0][1][:], ep2[:])
                                hold[0] = None
                                nc.tensor.matmul(s1bc[:], ones_t[:], q1[:],
                                                 start=(g == 1), stop=(g == ngrp - 1))
                                nc.tensor.matmul(s2bc[:], ones_t[:], q2[:],
                                                 start=(g == 1), stop=(g == ngrp - 1))
                            else:
                                nc.tensor.matmul(s1bc[:], ones_t[:], ep1[:],
                                                 start=(g == 0), stop=(g == ngrp - 1))
                                nc.tensor.matmul(s2bc[:], ones_t[:], ep2[:],
                                                 start=(g == 0), stop=(g == ngrp - 1))
                        for i, kc in enumerate(kcs_b):
                            j = kc - KCQ * qc
                            w0 = max(0, 128 * j)
                            if j >= 0:  # triangle mask on the diagonal block
                                nc.gpsimd.tensor_mul(
                                    e1[:, i, w0:w0 + 128], e1[:, i, w0:w0 + 128],
                                    msk_t[:, 0, 0:128])
                                nc.gpsimd.tensor_mul(
                                    e2[:, i, w0:w0 + 128], e2[:, i, w0:w0 + 128],
                                    msk_t[:, 0, 0:128])
                            st = (kc == 0)
                            sp = (kc == nkc - 1)
                            if not full_pair:
                                nc.tensor.matmul(s1bc[:, w0:512], ones_t[:],
                                                 e1[:, i, w0:512],
                                                 start=st, stop=sp)
                                nc.tensor.matmul(s2bc[:, w0:512], ones_t[:],
                                                 e2[:, i, w0:512],
                                                 start=st, stop=sp)
                            nc.tensor.matmul(
                                o1[:, w0:512], vb[:, kc, h * HD:(h + 1) * HD],
                                e1[:, i, w0:512], start=st, stop=sp)
                            nc.tensor.matmul(
                                o2[:, w0:512], vb[:, kc, h * HD:(h + 1) * HD],
                                e2[:, i, w0:512], start=st, stop=sp)

                    for g in range(ngrp):
                        kcs = [k for k in (2 * g, 2 * g + 1) if k < nkc]
                        s1g = psum.tile([128, 2, 512], F32, tag="sg", name="s1g", bufs=2)
                        s2g = psum.tile([128, 2, 512], F32, tag="sg", name="s2g", bufs=2)
                        for i, kc in enumerate(kcs):
                            j = kc - KCQ * qc
                            w0 = max(0, 128 * j)  # first valid col of chunk
                            nc.tensor.matmul(
                                s1g[:, i, w0:512], kt[h][0:64, kc * 128:(kc + 1) * 128],
                                qt[h][0:64, q0 + w0:q0 + 512], start=True, stop=True)
                            nc.tensor.matmul(
                                s2g[:, i, w0:512], kt[h][64:128, kc * 128:(kc + 1) * 128],
                                qt[h][64:128, q0 + w0:q0 + 512], start=True, stop=True)
                        e1 = ep.tile([128, 2, 512], BF16, tag="e1", name="e1", bufs=5)
                        e2 = ep.tile([128, 2, 512], BF16, tag="e2", name="e2", bufs=5)
                        nc.scalar.activation(
                            e1[:].rearrange("p a b -> p (a b)"),
                            s1g[:].rearrange("p a b -> p (a b)"),
                            AF.Exp, scale=float(SCALING))
                        nc.scalar.activation(
                            e2[:].rearrange("p a b -> p (a b)"),
                            s2g[:].rearrange("p a b -> p (a b)"),
                            AF.Exp, scale=float(SCALING))
                        pending.append((e1, e2, kcs, g))
                        if len(pending) > 2:
                            emit_b(pending.pop(0))
                    while pending:
                        emit_b(pending.pop(0))
                    # ---- epilogue: O = O1/s1 - lam*O2/s2 (divisions via ln/exp)
                    # free the psum accumulators ASAP: Ln reads sums, DVE
                    # copies drain the PV accumulators
                    aln = at.tile([128, 512], F32, tag="aln")
                    nc.scalar.activation(aln[:], s1bc[:], AF.Ln)
                    bln = at.tile([128, 512], F32, tag="bln")
                    nc.scalar.activation(bln[:], s2bc[:], AF.Ln)
                    o1s = at.tile([128, 512], F32, tag="o1s")
                    nc.vector.tensor_scalar(o1s[:], o1[:],
                                            lam_t[:, 3 * HPC + h:3 * HPC + h + 1],
                                            None, ALU.mult)
                    o2s = at.tile([128, 512], F32, tag="o2s")
                    nc.vector.tensor_copy(o2s[:], o2[:])
                    c = at.tile([128, 512], F32, tag="c")
                    nc.vector.tensor_sub(c[:], aln[:], bln[:])
                    u = at.tile([128, 512], F32, tag="u")
                    nc.scalar.activation(u[:], c[:], AF.Exp,
                                         bias=lam_t[:, HPC + h:HPC + h + 1])
                    nc.vector.tensor_scalar(u[:], u[:],
                                            lam_t[:, 2 * HPC + h:2 * HPC + h + 1],
                                            None, ALU.mult)
                    t = at.tile([128, 512], F32, tag="t")
                    nc.vector.tensor_mul(t[:], o2s[:], u[:])
                    d = at.tile([128, 512], BF16, tag="d")
                    nc.vector.tensor_sub(d[:], o1s[:], t[:])
                    osq = at.tile([128, 512], BF16, tag="osq")
                    nc.vector.tensor_mul(osq[:], d[:], d[:])
                    ssq = psum.tile([128, 512], F32, tag="s1bc", name="ssq")
                    nc.tensor.matmul(ssq[:], ones_t[:], osq[:],
                                     start=True, stop=True)
                    lnv = at.tile([128, 512], F32, tag="lnv")
                    nc.scalar.activation(lnv[:], ssq[:], AF.Ln,
                                         scale=float(1.0 / HD), bias=eps_t[:])
                    rr = at.tile([128, 512], BF16, tag="rr")
                    nc.scalar.activation(rr[:], lnv[:], AF.Exp, scale=-0.5)
                    nc.vector.tensor_mul(otf[h][:, q0:q0 + 512], d[:], rr[:])

        # ================= phase 3: output projection =================
        if True:
            for qc in range(QC):
                for oc in range(8):
                    ps = psum.tile([128, 2, 512], F32, tag="sg", name="ops", bufs=2)
                    for h in range(HPC):
                        nc.tensor.matmul(
                            ps[:, 0, :], wo_t[:, h, oc, :],
                            otf[h][:, qc * 512:(qc + 1) * 512],
                            start=(h == 0), stop=(h == HPC - 1))
                    ob = osb.tile([128, 512], F32, tag="ob")
                    nc.vector.tensor_copy(ob[:], ps[:, 0, :])
                    nc.sync.dma_start(
                        out_d[oc * 128:(oc + 1) * 128, qc * 512:(qc + 1) * 512],
                        ob[:])

    nc.compile()
    return nc


def _prep_inputs(X, Wq, Wk, Wv, Wo, lambda_q1, lambda_k1, lambda_q2,
                 lambda_k2, rms_scale):
    f32 = np.float32
    bf16 = ml_dtypes.bfloat16
    X = np.asarray(X, f32)
    Wq = np.asarray(Wq, f32)
    Wk = np.asarray(Wk, f32)
    Wv = np.asarray(Wv, f32)
    Wo = np.asarray(Wo, f32)
    lam = (np.exp(np.sum(np.asarray(lambda_q1, f32) * np.asarray(lambda_k1, f32), -1))
           - np.exp(np.sum(np.asarray(lambda_q2, f32) * np.asarray(lambda_k2, f32), -1))
           + f32(LAMBDA_INIT)).astype(f32)  # [H]
    # fold rms_scale and (1-lambda_init) into Wo
    wo_f = (Wo.reshape(H, HD, D_MODEL)
            * np.asarray(rms_scale, f32)[None, :, None]
            * f32(1.0 - LAMBDA_INIT)).astype(f32)

    # causal masks for the 4 diagonal-region chunk offsets
    msk = np.zeros((128, KCQ, 512), f32)
    kk = np.arange(128)[:, None]
    cc = np.arange(512)[None, :]
    for j in range(KCQ):
        msk[:, j, :] = (cc >= 128 * j + kk).astype(f32)

    in_maps = []
    for c in range(8):
        b, hg = divmod(c, 4)
        xt = X[b].T.reshape(MC, 128, N).transpose(1, 0, 2)  # [128, MC, N]
        sl = slice(hg * HPC * HD, (hg + 1) * HPC * HD)
        wq = Wq[:, sl].reshape(MC, 128, HPC * HD).transpose(1, 0, 2)
        wk = Wk[:, sl].reshape(MC, 128, HPC * HD).transpose(1, 0, 2)
        wv = Wv[:, sl].reshape(MC, 128, HPC * HD).transpose(1, 0, 2)
        wo = wo_f[hg * HPC:(hg + 1) * HPC].reshape(HPC, HD, 8, 128).transpose(1, 0, 2, 3)
        lv = lam[hg * HPC:(hg + 1) * HPC]
        g = np.maximum(np.abs(lv), f32(1.0)).astype(f32)
        with np.errstate(divide="ignore"):
            lnl = (np.log(np.abs(lv)) - np.log(g)).astype(f32)
        sgn = np.where(lv >= 0, f32(1.0), f32(-1.0))
        ginv = (f32(1.0) / g).astype(f32)
        lam_row = np.concatenate([lv, lnl, sgn, ginv]).astype(f32)
        lam_bc = np.broadcast_to(lam_row[None, :], (128, 4 * HPC))
        in_maps.append({
            "xt": np.ascontiguousarray(xt).astype(bf16),
            "wq": np.ascontiguousarray(wq).astype(bf16),
            "wk": np.ascontiguousarray(wk).astype(bf16),
            "wv": np.ascontiguousarray(wv).astype(bf16),
            "wo": np.ascontiguousarray(wo).astype(bf16),
            "lam": np.ascontiguousarray(lam_bc),
            "msk": msk.astype(bf16),
        })
    return in_maps


def kernel(X, Wq, Wk, Wv, Wo, lambda_q1, lambda_k1, lambda_q2, lambda_k2,
           rms_scale, _trace=False):
    if "nc" not in _cache:
        _cache["nc"] = _build()
    nc = _cache["nc"]
    in_maps = _prep_inputs(X, Wq, Wk, Wv, Wo, lambda_q1, lambda_k1,
                           lambda_q2, lambda_k2, rms_scale)
    res = run_bass_kernel_spmd(nc, in_maps, list(range(8)), trace=_trace)
    out = np.zeros((B, N, D_MODEL), np.float32)
    for c in range(8):
        b = c // 4
        out[b] += res.results[c]["outT"].T
    _cache["last_exec_ns"] = res.exec_time_ns
    _cache["last_res"] = res
    return out


# revision 20
# speedup vs baseline: 1.2013x; 1.2013x over previous
"""MultiHead Differential Attention on 8 Trainium2 NeuronCores.

Sharding: data-parallel over batch (B=2), tensor-parallel over heads
(16 heads -> 4 per core).  Core c handles batch c//4, heads (c%4)*4..+4.

Device layout is fully "transposed" (S^T = [k, q] orientation) so that no
on-device transposes are ever needed:
  - projections compute Q^T, K^T directly ([2*Dh, seq]); V naturally [seq, dv]
  - S^T[k,q] = K^T.T @ Q^T  (contraction over d on partitions)
  - softmax row-sums come from an all-ones stationary matmul (M=128 -> the
    sums arrive pre-broadcast across partitions)
  - PV keeps V stationary: O^T[dv,q] accumulates over k-chunks
  - combine/RMS happen on broadcast tiles; out-proj streams O^T with Wo
    stationary, producing OUT^T which the host transposes and sum-reduces.
"""
import numpy as np
import ml_dtypes
from contextlib import ExitStack

import concourse.bass as bass
import concourse.mybir as mybir
import concourse.tile as tile
from concourse import bacc
from concourse.bass_utils import run_bass_kernel_spmd

BF16 = mybir.dt.bfloat16
F32 = mybir.dt.float32
AF = mybir.ActivationFunctionType
ALU = mybir.AluOpType

D_MODEL = 1024
H = 16
DH = 64          # head dim per component
HD = 2 * DH      # 128, per-head width of Q/K/V
N = 2048         # sequence length
B = 2
HPC = 4          # heads per core
LAMBDA_INIT = 0.8
EPS = 1e-5
SCALING = 1.0 / np.sqrt(DH)

MC = D_MODEL // 128   # 8 contraction chunks for projections
QC = 4                # q chunks of 512
KCQ = 4               # k-chunks (128) per q chunk
NKC = 16              # total k chunks

_cache = {}


def _patch_act_tables():
    """Force Exp and Ln to resolve to the single set that contains both,
    so alternating Exp/Ln never reloads activation tables."""
    import concourse.bacc as bacc_mod
    import concourse.hw_specs as hw_specs_mod
    if getattr(bacc_mod, "_act_tables_patched", False):
        return
    orig = hw_specs_mod.get_activation_tables

    def patched(arch):
        t = orig(arch)
        for name, fns in t.items():
            if name != "natural_log_exp_and_others":
                fns.discard(AF.Exp)
                fns.discard(AF.Ln)
        return t

    bacc_mod.get_activation_tables = patched
    bacc_mod._act_tables_patched = True


def _build():
    _patch_act_tables()
    nc = bacc.Bacc("TRN2", target_bir_lowering=False, debug=False)

    xt_d = nc.dram_tensor("xt", [128, MC, N], BF16, kind="ExternalInput").ap()
    wq_d = nc.dram_tensor("wq", [128, MC, HPC * HD], BF16, kind="ExternalInput").ap()
    wk_d = nc.dram_tensor("wk", [128, MC, HPC * HD], BF16, kind="ExternalInput").ap()
    wv_d = nc.dram_tensor("wv", [128, MC, HPC * HD], BF16, kind="ExternalInput").ap()
    wo_d = nc.dram_tensor("wo", [128, HPC, 8, 128], BF16, kind="ExternalInput").ap()
    lam_d = nc.dram_tensor("lam", [128, 4 * HPC], F32, kind="ExternalInput").ap()
    msk_d = nc.dram_tensor("msk", [128, KCQ, 512], BF16, kind="ExternalInput").ap()
    out_d = nc.dram_tensor("outT", [D_MODEL, N], F32, kind="ExternalOutput").ap()

    with tile.TileContext(nc) as tc, ExitStack() as ctx:
        # ---- long-lived tiles
        keep = ctx.enter_context(tc.tile_pool(name="keep", bufs=1))
        qt = [keep.tile([128, N], BF16, tag=f"qt{h}", name=f"qt{h}") for h in range(HPC)]
        kt = [keep.tile([128, N], BF16, tag=f"kt{h}", name=f"kt{h}") for h in range(HPC)]
        vb = keep.tile([128, NKC, 512], BF16, tag="vb")
        otf = [keep.tile([128, N], BF16, tag=f"otf{h}", name=f"otf{h}") for h in range(HPC)]
        lam_t = keep.tile([128, 4 * HPC], F32, tag="lam")
        msk_t = keep.tile([128, KCQ, 512], BF16, tag="msk")
        ones_t = keep.tile([128, 128], BF16, tag="ones")
        eps_t = keep.tile([128, 1], F32, tag="eps")
        wo_t = keep.tile([128, HPC, 8, 128], BF16, tag="wo")

        nc.sync.dma_start(lam_t[:], lam_d[:])
        nc.sync.dma_start(msk_t[:], msk_d[:])
        nc.sync.dma_start(wo_t[:], wo_d[:])
        nc.gpsimd.memset(ones_t[:], 1.0)
        nc.gpsimd.memset(eps_t[:], float(EPS))

        # ================= phase 1: projections =================
        pj = ctx.enter_context(tc.tile_pool(name="proj", bufs=1))
        psum = ctx.enter_context(tc.tile_pool(name="psum", bufs=1, space="PSUM"))
        at = ctx.enter_context(tc.tile_pool(name="att", bufs=2))
        ep = ctx.enter_context(tc.tile_pool(name="esb", bufs=2))
        osb = ctx.enter_context(tc.tile_pool(name="osb", bufs=2))
        if True:
            xtb = pj.tile([128, MC, N], BF16, tag="xtb")
            wqb = pj.tile([128, MC, HPC * HD], BF16, tag="wqb")
            wkb = pj.tile([128, MC, HPC * HD], BF16, tag="wkb")
            wvb = pj.tile([128, MC, HPC * HD], BF16, tag="wvb")
            for mc in range(MC):
                nc.sync.dma_start(xtb[:, mc, :], xt_d[:, mc, :])
                nc.sync.dma_start(wvb[:, mc, :], wv_d[:, mc, :])
                nc.sync.dma_start(wqb[:, mc, :], wq_d[:, mc, :])
                nc.sync.dma_start(wkb[:, mc, :], wk_d[:, mc, :])

            # interleaved: V seq-chunks for qc, then all heads' QT/KT qc-chunk,
            # so attention(qc=0) unblocks as early as possible
            for qc in range(QC):
                for sc in range(4 * qc, 4 * qc + 4):
                    ps = psum.tile([128, 2, 512], F32, tag="sg", name="vps", bufs=2)
                    for mc in range(MC):
                        nc.tensor.matmul(
                            ps[:, 0, :],
                            xtb[:, mc, sc * 128:(sc + 1) * 128],
                            wvb[:, mc, :],
                            start=(mc == 0), stop=(mc == MC - 1))
                    nc.scalar.copy(vb[:, sc, :], ps[:, 0, :])
                for h in range(HPC):
                    for (wsrc, dst) in ((wqb, qt[h]), (wkb, kt[h])):
                        ps = psum.tile([128, 2, 512], F32, tag="sg", name="qkps", bufs=2)
                        for mc in range(MC):
                            nc.tensor.matmul(
                                ps[:, 0, :],
                                wsrc[:, mc, h * HD:(h + 1) * HD],
                                xtb[:, mc, qc * 512:(qc + 1) * 512],
                                start=(mc == 0), stop=(mc == MC - 1))
                        nc.scalar.copy(dst[:, qc * 512:(qc + 1) * 512], ps[:, 0, :])

        # ================= phase 2: attention =================
        if True:
            for qc in range(QC):
                for h in range(HPC):
                    nkc = KCQ * qc + KCQ  # k chunks in play
                    q0 = qc * 512
                    s1bc = psum.tile([128, 512], F32, tag="s1bc")
                    s2bc = psum.tile([128, 512], F32, tag="s2bc")
                    o1 = psum.tile([128, 512], F32, tag="o1")
                    o2 = psum.tile([128, 512], F32, tag="o2")
                    ngrp = (nkc + 1) // 2
                    hold = [None]
                    pending = []

                    def emit_b(item):
                        e1, e2, kcs_b, g = item
                        full_pair = (2 * g + 1 < KCQ * qc)
                        if full_pair:
                            ep1 = ep.tile([128, 512], BF16, tag="ep1", name="ep1", bufs=3)
                            nc.vector.tensor_add(ep1[:], e1[:, 0, :], e1[:, 1, :])
                            ep2 = ep.tile([128, 512], BF16, tag="ep2", name="ep2", bufs=3)
                            nc.vector.tensor_add(ep2[:], e2[:, 0, :], e2[:, 1, :])
                            if g % 2 == 0 and 2 * (g + 1) + 1 < KCQ * qc:
                                hold[0] = (ep1, ep2)  # fold into partner pair
                            elif g % 2 == 1 and hold[0] is not None:
                                q1 = ep.tile([128, 512], BF16, tag="q1", name="q1")
                                nc.vector.tensor_add(q1[:], hold[0][0][:], ep1[:])
                                q2 = ep.tile([128, 512], BF16, tag="q2", name="q2")
                                nc.vector.tensor_add(q2[:], hold[0][1][:], ep2[:])
                                hold[0] = None
                                nc.tensor.matmul(s1bc[:], ones_t[:], q1[:],
                                                 start=(g == 1), stop=(g == ngrp - 1))
                                nc.tensor.matmul(s2bc[:], ones_t[:], q2[:],
                                                 start=(g == 1), stop=(g == ngrp - 1))
                            else:
                                nc.tensor.matmul(s1bc[:], ones_t[:], ep1[:],
                                                 start=(g == 0), stop=(g == ngrp - 1))
                                nc.tensor.matmul(s2bc[:], ones_t[:], ep2[:],
                                                 start=(g == 0), stop=(g == ngrp - 1))
                        for i, kc in enumerate(kcs_b):
                            j = kc - KCQ * qc
                            w0 = max(0, 128 * j)
                            if j >= 0:  # triangle mask on the diagonal block
                                nc.vector.tensor_mul(
                                    e1[:, i, w0:w0 + 128], e1[:, i, w0:w0 + 128],
                                    msk_t[:, 0, 0:128])
                                nc.vector.tensor_mul(
                                    e2[:, i, w0:w0 + 128], e2[:, i, w0:w0 + 128],
                                    msk_t[:, 0, 0:128])
                            st = (kc == 0)
                            sp = (kc == nkc - 1)
                            if not full_pair:
                                nc.tensor.matmul(s1bc[:, w0:512], ones_t[:],
                                                 e1[:, i, w0:512],
                                                 start=st, stop=sp)
                                nc.tensor.matmul(s2bc[:, w0:512], ones_t[:],
                                                 e2[:, i, w0:512],
                                                 start=st, stop=sp)
                            nc.tensor.matmul(
                                o1[:, w0:512], vb[:, kc, h * HD:(h + 1) * HD],
                                e1[:, i, w0:512], start=st, stop=sp)
                            nc.tensor.matmul(
                                o2[:, w0:512], vb[:, kc, h * HD:(h + 1) * HD],
                                e2[:, i, w0:512], start=st, stop=sp)

                    for g in range(ngrp):
                        kcs = [k for k in (2 * g, 2 * g + 1) if k < nkc]
                        s1g = psum.tile([128, 2, 512], F32, tag="sg", name="s1g", bufs=2)
                        s2g = psum.tile([128, 2, 512], F32, tag="sg", name="s2g", bufs=2)
                        for i, kc in enumerate(kcs):
                            j = kc - KCQ * qc
                            w0 = max(0, 128 * j)  # first valid col of chunk
                            nc.tensor.matmul(
                                s1g[:, i, w0:512], kt[h][0:64, kc * 128:(kc + 1) * 128],
                                qt[h][0:64, q0 + w0:q0 + 512], start=True, stop=True)
                            nc.tensor.matmul(
                                s2g[:, i, w0:512], kt[h][64:128, kc * 128:(kc + 1) * 128],
                                qt[h][64:128, q0 + w0:q0 + 512], start=True, stop=True)
                        e1 = ep.tile([128, 2, 512], BF16, tag="e1", name="e1", bufs=5)
                        e2 = ep.tile([128, 2, 512], BF16, tag="e2", name="e2", bufs=5)
                        nc.scalar.activation(
                            e1[:].rearrange("p a b -> p (a b)"),
                            s1g[:].rearrange("p a b -> p (a b)"),
                            AF.Exp, scale=float(SCALING))
                        nc.scalar.activation(
                            e2[:].rearrange("p a b -> p (a b)"),
                            s2g[:].rearrange("p a b -> p (a b)"),
                            AF.Exp, scale=float(SCALING))
                        pending.append((e1, e2, kcs, g))
                        if len(pending) > 2:
                            emit_b(pending.pop(0))
                    while pending:
                        emit_b(pending.pop(0))
                    # ---- epilogue: O = O1/s1 - lam*O2/s2 (divisions via ln/exp)
                    # free the psum accumulators ASAP: Ln reads sums, DVE
                    # copies drain the PV accumulators
                    aln = at.tile([128, 512], F32, tag="aln")
                    nc.scalar.activation(aln[:], s1bc[:], AF.Ln)
                    bln = at.tile([128, 512], F32, tag="bln")
                    nc.scalar.activation(bln[:], s2bc[:], AF.Ln)
                    o1s = at.tile([128, 512], F32, tag="o1s")
                    nc.vector.tensor_scalar(o1s[:], o1[:],
                                            lam_t[:, 3 * HPC + h:3 * HPC + h + 1],
                                            None, ALU.mult)
                    o2s = at.tile([128, 512], F32, tag="o2s")
                    nc.vector.tensor_copy(o2s[:], o2[:])
                    c = at.tile([128, 512], F32, tag="c")
                    nc.vector.tensor_sub(c[:], aln[:], bln[:])
                    u = at.tile([128, 512], F32, tag="u")
                    nc.scalar.activation(u[:], c[:], AF.Exp,
                                         bias=lam_t[:, HPC + h:HPC + h + 1])
                    nc.vector.tensor_scalar(u[:], u[:],
                                            lam_t[:, 2 * HPC + h:2 * HPC + h + 1],
                                            None, ALU.mult)
                    t = at.tile([128, 512], F32, tag="t")
                    nc.vector.tensor_mul(t[:], o2s[:], u[:])
                    d = at.tile([128, 512], BF16, tag="d")
                    nc.vector.tensor_sub(d[:], o1s[:], t[:])
                    osq = at.tile([128, 512], BF16, tag="osq")
                    nc.vector.tensor_mul(osq[:], d[:], d[:])
                    ssq = psum.tile([128, 512], F32, tag="s1bc", name="ssq")
                    nc.tensor.matmul(ssq[:], ones_t[:], osq[:],
                                     start=True, stop=True)
                    lnv = at.tile([128, 512], F32, tag="lnv")
                    nc.scalar.activation(lnv[:], ssq[:], AF.Ln,
                                         scale=float(1.0 / HD), bias=eps_t[:])
                    rr = at.tile([128, 512], BF16, tag="rr")
                    nc.scalar.activation(rr[:], lnv[:], AF.Exp, scale=-0.5)
                    nc.vector.tensor_mul(otf[h][:, q0:q0 + 512], d[:], rr[:])

        # ================= phase 3: output projection =================
        if True:
            for qc in range(QC):
                for oc in range(8):
                    ps = psum.tile([128, 2, 512], F32, tag="sg", name="ops", bufs=2)
                    for h in range(HPC):
                        nc.tensor.matmul(
                            ps[:, 0, :], wo_t[:, h, oc, :],
                            otf[h][:, qc * 512:(qc + 1) * 512],
                            start=(h == 0), stop=(h == HPC - 1))
                    ob = osb.tile([128, 512], F32, tag="ob")
                    nc.vector.tensor_copy(ob[:], ps[:, 0, :])
                    nc.sync.dma_start(
                        out_d[oc * 128:(oc + 1) * 128, qc * 512:(qc + 1) * 512],
                        ob[:])

    nc.compile()
    return nc


def _prep_inputs(X, Wq, Wk, Wv, Wo, lambda_q1, lambda_k1, lambda_q2,
                 lambda_k2, rms_scale):
    f32 = np.float32
    bf16 = ml_dtypes.bfloat16
    X = np.asarray(X, f32)
    Wq = np.asarray(Wq, f32)
    Wk = np.asarray(Wk, f32)
    Wv = np.asarray(Wv, f32)
    Wo = np.asarray(Wo, f32)
    lam = (np.exp(np.sum(np.asarray(lambda_q1, f32) * np.asarray(lambda_k1, f32), -1))
           - np.exp(np.sum(np.asarray(lambda_q2, f32) * np.asarray(lambda_k2, f32), -1))
           + f32(LAMBDA_INIT)).astype(f32)  # [H]
    # fold rms_scale and (1-lambda_init) into Wo
    wo_f = (Wo.reshape(H, HD, D_MODEL)
            * np.asarray(rms_scale, f32)[None, :, None]
            * f32(1.0 - LAMBDA_INIT)).astype(f32)

    # causal masks for the 4 diagonal-region chunk offsets
    msk = np.zeros((128, KCQ, 512), f32)
    kk = np.arange(128)[:, None]
    cc = np.arange(512)[None, :]
    for j in range(KCQ):
        msk[:, j, :] = (cc >= 128 * j + kk).astype(f32)

    in_maps = []
    for c in range(8):
        b, hg = divmod(c, 4)
        xt = X[b].T.reshape(MC, 128, N).transpose(1, 0, 2)  # [128, MC, N]
        sl = slice(hg * HPC * HD, (hg + 1) * HPC * HD)
        wq = Wq[:, sl].reshape(MC, 128, HPC * HD).transpose(1, 0, 2)
        wk = Wk[:, sl].reshape(MC, 128, HPC * HD).transpose(1, 0, 2)
        wv = Wv[:, sl].reshape(MC, 128, HPC * HD).transpose(1, 0, 2)
        wo = wo_f[hg * HPC:(hg + 1) * HPC].reshape(HPC, HD, 8, 128).transpose(1, 0, 2, 3)
        lv = lam[hg * HPC:(hg + 1) * HPC]
        g = np.maximum(np.abs(lv), f32(1.0)).astype(f32)
        with np.errstate(divide="ignore"):
            lnl = (np.log(np.abs(lv)) - np.log(g)).astype(f32)
        sgn = np.where(lv >= 0, f32(1.0), f32(-1.0))
        ginv = (f32(1.0) / g).astype(f32)
        lam_row = np.concatenate([lv, lnl, sgn, ginv]).astype(f32)
        lam_bc = np.broadcast_to(lam_row[None, :], (128, 4 * HPC))
        in_maps.append({
            "xt": np.ascontiguousarray(xt).astype(bf16),
            "wq": np.ascontiguousarray(wq).astype(bf16),
            "wk": np.ascontiguousarray(wk).astype(bf16),
            "wv": np.ascontiguousarray(wv).astype(bf16),
            "wo": np.ascontiguousarray(wo).astype(bf16),
            "lam": np.ascontiguousarray(lam_bc),
            "msk": msk.astype(bf16),
        })
    return in_maps


def kernel(X, Wq, Wk, Wv, Wo, lambda_q1, lambda_k1, lambda_q2, lambda_k2,
           rms_scale, _trace=False):
    if "nc" not in _cache:
        _cache["nc"] = _build()
    nc = _cache["nc"]
    in_maps = _prep_inputs(X, Wq, Wk, Wv, Wo, lambda_q1, lambda_k1,
                           lambda_q2, lambda_k2, rms_scale)
    res = run_bass_kernel_spmd(nc, in_maps, list(range(8)), trace=_trace)
    out = np.zeros((B, N, D_MODEL), np.float32)
    for c in range(8):
        b = c // 4
        out[b] += res.results[c]["outT"].T
    _cache["last_exec_ns"] = res.exec_time_ns
    _cache["last_res"] = res
    return out
